# revision 10
# baseline (speedup 1.0000x reference)
"""Trainium2 Bass kernel for nn_LionCrossAttentionDimH.

Computes, per (b, h) slice (H treated as independent "heads"):
    x = LN(hidden); e = LN(enc)
    q = x@Wq + bq ; k = e@Wk + bk ; v = e@Wv + bv        [48, 512]
    attn = softmax((q^T k) * sqrt(1/48), axis=c)          [512, 512]
    out  = (v @ attn) @ Wo + bo + hidden                  [48, 512]
Returns (out, attn) with full shapes [8,48,48,512] and [8,48,512,512].

Sharding: data-parallel over batch B=8 -> 8 NeuronCores (SPMD, no
collectives). Weights replicated; LN gains and the attn scale are folded
into the projection weights on the host; biases are applied on-chip via
K=1 ones-matmuls accumulated in PSUM.

Layout: h-slices are processed in pairs. Pair tensors use 112 partition
rows with slice0 at rows 0-47 and slice1 at rows 64-111 (pad rows 48-63
zeroed) so every per-slice engine access starts at partition 0 or 64 —
hardware requires 32-aligned partition starts.
"""

import math
import os
import sys
from contextlib import ExitStack

for _p in ("/opt/trn_rl_repo",):
    if _p not in sys.path:
        sys.path.insert(0, _p)

import numpy as np

import concourse.bass as bass
import concourse.tile as tile
from concourse import bacc, mybir
from concourse.bass import ds, ts
from concourse.bass_utils import run_bass_kernel_spmd
from concourse.masks import make_identity

# ---- problem constants (hardcoded per contract) ----
B, H, W = 8, 48, 48
HID, ENC = 512, 768
N_CORES = 8
P = 128
HID_C = HID // P   # 4
ENC_C = ENC // P   # 6
EPS = 1e-5
PW = 112           # padded pair width (rows 0-47 slice0, 64-111 slice1)

F32 = mybir.dt.float32
F32R = mybir.dt.float32r

# dtype knob for the big matmuls: "f32" (exact, 4 cyc/col) or
# "f32r" (single-pass, 1 cyc/col at N>=256).
MM_MODE = os.environ.get("KERNEL_MM", "f32r")
MMDT = F32R if MM_MODE == "f32r" else F32

LAST_RESULTS = None  # BassKernelResults of the most recent kernel() call


def _dview(ap):
    """Bitcast a DRAM-side fp32 AP to the matmul dtype (same bytes)."""
    if MM_MODE == "f32r":
        return ap.bitcast(F32R)
    return ap


def build_program():
    nc = bacc.Bacc(
        "TRN2", target_bir_lowering=False, debug=False, num_devices=N_CORES
    )

    hid_d = nc.dram_tensor("hidden", [H, W, HID], F32, kind="ExternalInput").ap()
    enc_d = nc.dram_tensor("enc", [H, W, ENC], F32, kind="ExternalInput").ap()
    wq_d = nc.dram_tensor("wq", [HID, HID], F32, kind="ExternalInput").ap()
    wk_d = nc.dram_tensor("wk", [ENC, HID], F32, kind="ExternalInput").ap()
    wv_d = nc.dram_tensor("wv", [ENC, HID], F32, kind="ExternalInput").ap()
    wo_d = nc.dram_tensor("wo", [HID, HID], F32, kind="ExternalInput").ap()
    bias_d = nc.dram_tensor("biases", [4, HID], F32, kind="ExternalInput").ap()

    out_d = nc.dram_tensor("out", [H, W, HID], F32, kind="ExternalOutput").ap()
    attn_d = nc.dram_tensor("attn", [H, HID, HID], F32, kind="ExternalOutput").ap()

    with tile.TileContext(nc) as tc, ExitStack() as ctx:
        singles = ctx.enter_context(tc.tile_pool(name="singles", bufs=1))
        io = ctx.enter_context(tc.tile_pool(name="io", bufs=3))
        work = ctx.enter_context(tc.tile_pool(name="work", bufs=2))
        epool = ctx.enter_context(tc.tile_pool(name="epool", bufs=3))
        small = ctx.enter_context(tc.tile_pool(name="small", bufs=4))
        stats = ctx.enter_context(tc.tile_pool(name="stats", bufs=4))
        ps_t = ctx.enter_context(tc.tile_pool(name="ps_t", bufs=2, space="PSUM"))
        ps_big = ctx.enter_context(tc.tile_pool(name="ps_big", bufs=3, space="PSUM"))
        ps_a = ctx.enter_context(tc.tile_pool(name="ps_a", bufs=2, space="PSUM"))
        ps_s = ctx.enter_context(tc.tile_pool(name="ps_s", bufs=1, space="PSUM"))

        # ---- constants / weights resident in SBUF ----
        ident = singles.tile([P, P], F32)
        make_identity(nc, ident)
        ones_cf = singles.tile([P, 1], F32)
        nc.vector.memset(ones_cf, 1.0)
        ones_col = singles.tile([P, 1], MMDT)
        nc.scalar.copy(out=ones_col, in_=ones_cf)
        ones_rf = singles.tile([1, P], F32)
        nc.vector.memset(ones_rf, 1.0)
        ones_row = singles.tile([1, P], MMDT)
        nc.scalar.copy(out=ones_row, in_=ones_rf)
        zeros_t = singles.tile([P, 64], F32)
        nc.vector.memset(zeros_t, 0.0)
        eps_t = singles.tile([P, 1], F32)
        nc.vector.memset(eps_t, EPS)

        wq_sb = singles.tile([P, HID_C, HID], MMDT)
        nc.sync.dma_start(wq_sb, _dview(wq_d.rearrange("(o p) n -> p o n", p=P)))
        wk_sb = singles.tile([P, ENC_C, HID], MMDT)
        nc.sync.dma_start(wk_sb, _dview(wk_d.rearrange("(o p) n -> p o n", p=P)))
        wv_sb = singles.tile([P, ENC_C, HID], MMDT)
        nc.sync.dma_start(wv_sb, _dview(wv_d.rearrange("(o p) n -> p o n", p=P)))
        wo_sb = singles.tile([P, HID_C, HID], MMDT)
        nc.sync.dma_start(wo_sb, _dview(wo_d.rearrange("(o p) n -> p o n", p=P)))
        bias_sb = singles.tile([1, 4, HID], MMDT)
        nc.sync.dma_start(bias_sb, _dview(bias_d[None, :, :]))

        def load_pair(dram, h0, d, tag):
            """Load dram[h0] -> rows 0-47, dram[h0+1] -> rows 64-111."""
            t = io.tile([PW, d], F32, tag=tag)
            nc.gpsimd.memset(t[32:64], 0.0)
            nc.sync.dma_start(t[0:48], dram[h0].rearrange("w d -> w d"))
            nc.sync.dma_start(t[64:112], dram[h0 + 1].rearrange("w d -> w d"))
            return t

        def layernorm(x_t, d, tag):
            """x_t: [PW, d] raw input -> returns normalized [PW, d] tile."""
            if d <= 512:
                st = stats.tile([PW, 6], F32, tag=f"st_{tag}")
                nc.vector.bn_stats(out=st, in_=x_t)
            else:
                gs = math.gcd(512, d)
                ng = d // gs
                st = stats.tile([PW, ng, 6], F32, tag=f"st_{tag}")
                for i in range(ng):
                    nc.vector.bn_stats(out=st[:, i, :], in_=x_t[:, ts(i, gs)])
            mv = stats.tile([PW, 2], F32, tag=f"mv_{tag}")
            nc.vector.bn_aggr(out=mv, in_=st)
            rstd = stats.tile([PW, 1], F32, tag=f"rstd_{tag}")
            # rstd = 1/sqrt(var + eps)
            nc.scalar.activation(
                out=rstd, in_=mv[:, 1:2], func=mybir.ActivationFunctionType.Sqrt,
                bias=eps_t[:PW], scale=1.0,
            )
            nc.vector.reciprocal(out=rstd, in_=rstd)
            x_ln = work.tile([PW, d], F32, tag=f"ln_{tag}")
            nc.gpsimd.tensor_scalar(
                out=x_ln, in0=x_t, scalar1=mv[:, 0:1], scalar2=rstd,
                op0=mybir.AluOpType.subtract, op1=mybir.AluOpType.mult,
            )
            return x_ln

        def transpose_pair(x_ln, nchunk, tag, dt=None):
            """[PW, 128*nchunk] -> [128, nchunk, PW] via PE transposes."""
            xT = work.tile([P, nchunk, PW], dt or MMDT, tag=f"T_{tag}")
            for m in range(nchunk):
                pt = ps_t.tile([P, PW], F32, tag="pst")
                nc.tensor.transpose(pt, x_ln[:, ts(m, P)], ident[:PW, :PW])
                nc.scalar.copy(out=xT[:, m, :], in_=pt)
            return xT

        def project(xT, w_sb, nchunk, bias_idx, tag):
            """pair projection: [PW,512] tile = xT.T @ W + bias."""
            pq = ps_big.tile([PW, HID], F32, tag="ps_big")
            for m in range(nchunk):
                nc.tensor.matmul(
                    pq, xT[:, m, :], w_sb[:, m, :],
                    start=(m == 0), stop=False,
                )
            # bias add: [1,PW] ones lhsT x [1,512] bias rhs
            nc.tensor.matmul(
                pq, ones_row[:, :PW], bias_sb[:, bias_idx, :],
                start=False, stop=True,
            )
            o = work.tile([PW, HID], MMDT, tag=f"proj_{tag}")
            nc.vector.tensor_copy(out=o, in_=pq)
            return o

        for j in range(H // 2):
            h0 = 2 * j
            x_t = load_pair(hid_d, h0, HID, "x_t")
            e_t = load_pair(enc_d, h0, ENC, "e_t")

            x_ln = layernorm(x_t, HID, "x")
            e_ln = layernorm(e_t, ENC, "e")
            xT = transpose_pair(x_ln, HID_C, "x")
            eT = transpose_pair(e_ln, ENC_C, "e")

            q_sb = project(xT, wq_sb, HID_C, 0, "q")
            k_sb = project(eT, wk_sb, ENC_C, 1, "k")
            v_sb = project(eT, wv_sb, ENC_C, 2, "v")
            vT = transpose_pair(v_sb.bitcast(F32), HID_C, "v")

            uT = work.tile([P, HID_C, PW], MMDT, tag="T_u")
            nc.scalar.copy(
                out=uT[:, :, 48:64],
                in_=zeros_t.rearrange("p (a b) -> p a b", a=HID_C),
            )
            for s in range(2):
                h = h0 + s
                sb = 64 * s
                E_sb = epool.tile([P, HID_C, HID], MMDT, tag="E")
                for m in range(HID_C):
                    pa = ps_a.tile([P, HID], F32, tag="ps_attn")
                    nc.tensor.matmul(
                        pa, q_sb[ds(sb, 48), ts(m, P)],
                        k_sb[ds(sb, 48), :],
                        start=True, stop=True,
                    )
                    nc.scalar.activation(
                        out=E_sb[:, m, :], in_=pa,
                        func=mybir.ActivationFunctionType.Exp,
                    )
                # column sums (over the partition/c axis) via ones-matmul
                pss = ps_s.tile([1, HID], F32, tag="ps_s")
                for m in range(HID_C):
                    nc.tensor.matmul(
                        pss, ones_col, E_sb[:, m, :],
                        start=(m == 0), stop=(m == HID_C - 1),
                    )
                s_sb = small.tile([1, HID], MMDT, tag="s_sb")
                nc.vector.tensor_copy(out=s_sb, in_=pss)
                # broadcast sums to 128 partitions, then reciprocal
                pr = ps_a.tile([P, HID], F32, tag="ps_attn")
                nc.tensor.matmul(
                    pr, ones_row[:1, :], s_sb, start=True, stop=True
                )
                r_sb = epool.tile([P, HID], F32, tag="r_sb")
                nc.vector.reciprocal(out=r_sb, in_=pr)
                # normalize E in place (softmax denominator)
                for m in range(HID_C):
                    nc.any.tensor_tensor(
                        out=E_sb[:, m, :], in0=E_sb[:, m, :], in1=r_sb,
                        op=mybir.AluOpType.mult,
                    )
                # attn output to DRAM
                nc.sync.dma_start(
                    _dview(attn_d[h].rearrange("(o p) n -> p o n", p=P)), E_sb
                )
                # u = v @ attn   [48, 512]
                pu = ps_big.tile([PW, HID], F32, tag="ps_big")
                for m in range(HID_C):
                    nc.tensor.matmul(
                        pu[:48], vT[:, m, ds(sb, 48)], E_sb[:, m, :],
                        start=(m == 0), stop=(m == HID_C - 1),
                    )
                u_s = work.tile([48, HID], F32, tag=f"u_s{s}")
                nc.vector.tensor_copy(out=u_s, in_=pu[:48])
                for m in range(HID_C):
                    pt = ps_t.tile([P, PW], F32, tag="pst")
                    nc.tensor.transpose(
                        pt[:, :48], u_s[:, ts(m, P)], ident[:48, :48]
                    )
                    nc.scalar.copy(out=uT[:, m, ds(sb, 48)], in_=pt[:, :48])

            po = ps_big.tile([PW, HID], F32, tag="ps_big")
            for m in range(HID_C):
                nc.tensor.matmul(
                    po, uT[:, m, :], wo_sb[:, m, :],
                    start=(m == 0), stop=False,
                )
            nc.tensor.matmul(
                po, ones_row[:, :PW], bias_sb[:, 3, :],
                start=False, stop=True,
            )
            o_sb = work.tile([PW, HID], F32, tag="o_sb")
            nc.any.tensor_tensor(
                out=o_sb, in0=po, in1=x_t, op=mybir.AluOpType.add
            )
            nc.sync.dma_start(out_d[h0], o_sb[0:48])
            nc.sync.dma_start(out_d[h0 + 1], o_sb[64:112])

    nc.compile()
    return nc


def kernel(**inputs):
    global LAST_RESULTS
    hidden = np.asarray(inputs["hidden"], dtype=np.float32)
    enc = np.asarray(inputs["enc"], dtype=np.float32)
    ln_g = np.asarray(inputs["ln_g"], dtype=np.float32)
    ln_b = np.asarray(inputs["ln_b"], dtype=np.float32)
    eln_g = np.asarray(inputs["eln_g"], dtype=np.float32)
    eln_b = np.asarray(inputs["eln_b"], dtype=np.float32)
    Wq = np.asarray(inputs["Wq"], dtype=np.float32)
    bq = np.asarray(inputs["bq"], dtype=np.float32)
    Wk = np.asarray(inputs["Wk"], dtype=np.float32)
    bk = np.asarray(inputs["bk"], dtype=np.float32)
    Wv = np.asarray(inputs["Wv"], dtype=np.float32)
    bv = np.asarray(inputs["bv"], dtype=np.float32)
    Wo = np.asarray(inputs["Wo"], dtype=np.float32)
    bo = np.asarray(inputs["bo"], dtype=np.float32)

    scale = math.sqrt(1.0 / W)
    # fold LN affine + attn scale into the projections (exact for g=1, b=0)
    wq_eff = (ln_g[:, None] * Wq) * scale
    bq_eff = (ln_b @ Wq + bq) * scale
    wk_eff = eln_g[:, None] * Wk
    bk_eff = eln_b @ Wk + bk
    wv_eff = eln_g[:, None] * Wv
    bv_eff = eln_b @ Wv + bv
    biases = np.stack([bq_eff, bk_eff, bv_eff, bo]).astype(np.float32)

    nc = build_program()

    in_maps = []
    for b in range(N_CORES):
        in_maps.append({
            "hidden": np.ascontiguousarray(hidden[b]),
            "enc": np.ascontiguousarray(enc[b]),
            "wq": np.ascontiguousarray(wq_eff),
            "wk": np.ascontiguousarray(wk_eff),
            "wv": np.ascontiguousarray(wv_eff),
            "wo": np.ascontiguousarray(Wo),
            "biases": biases,
        })

    res = run_bass_kernel_spmd(
        nc, in_maps, core_ids=list(range(N_CORES)),
        trace=bool(os.environ.get("KERNEL_TRACE")),
    )
    LAST_RESULTS = res

    out = np.stack([res.results[b]["out"] for b in range(N_CORES)])
    attn = np.stack([res.results[b]["attn"] for b in range(N_CORES)])
    return out, attn


# revision 14
# speedup vs baseline: 1.4645x; 1.4645x over previous
"""Trainium2 Bass kernel for nn_LionCrossAttentionDimH.

Computes, per (b, h) slice (H treated as independent "heads"):
    x = LN(hidden); e = LN(enc)
    q = x@Wq + bq ; k = e@Wk + bk ; v = e@Wv + bv        [48, 512]
    attn = softmax((q^T k) * sqrt(1/48), axis=c)          [512, 512]
    out  = (v @ attn) @ Wo + bo + hidden                  [48, 512]
Returns (out, attn) with full shapes [8,48,48,512] and [8,48,512,512].

Sharding: data-parallel over batch B=8 -> 8 NeuronCores (SPMD, no
collectives). Weights replicated; LN gains and the attn scale are folded
into the projection weights on the host; biases are applied on-chip via
K=1 ones-matmuls accumulated in PSUM.

Layout: h-slices are processed in pairs. Pair tensors use 112 partition
rows with slice0 at rows 0-47 and slice1 at rows 64-111 (pad rows 48-63
zeroed) so every per-slice engine access starts at partition 0 or 64 —
hardware requires 32-aligned partition starts.
"""

import math
import os
import sys
from contextlib import ExitStack

for _p in ("/opt/trn_rl_repo",):
    if _p not in sys.path:
        sys.path.insert(0, _p)

import numpy as np

import concourse.bass as bass
import concourse.tile as tile
from concourse import bacc, mybir
from concourse.bass import ds, ts
from concourse.bass_utils import run_bass_kernel_spmd
from concourse.masks import make_identity

# ---- problem constants (hardcoded per contract) ----
B, H, W = 8, 48, 48
HID, ENC = 512, 768
N_CORES = 8
P = 128
HID_C = HID // P   # 4
ENC_C = ENC // P   # 6
EPS = 1e-5
PW = 112           # padded pair width (rows 0-47 slice0, 64-111 slice1)

F32 = mybir.dt.float32
F32R = mybir.dt.float32r

# dtype knob for the big matmuls: "f32" (exact, 4 cyc/col) or
# "f32r" (single-pass, 1 cyc/col at N>=256).
MM_MODE = os.environ.get("KERNEL_MM", "f32r")
MMDT = F32R if MM_MODE == "f32r" else F32

LAST_RESULTS = None  # BassKernelResults of the most recent kernel() call


def _dview(ap):
    """Bitcast a DRAM-side fp32 AP to the matmul dtype (same bytes)."""
    if MM_MODE == "f32r":
        return ap.bitcast(F32R)
    return ap


def build_program():
    nc = bacc.Bacc(
        "TRN2", target_bir_lowering=False, debug=False, num_devices=N_CORES
    )

    hid_d = nc.dram_tensor("hidden", [H, W, HID], F32, kind="ExternalInput").ap()
    enc_d = nc.dram_tensor("enc", [H, W, ENC], F32, kind="ExternalInput").ap()
    wq_d = nc.dram_tensor("wq", [HID, HID], F32, kind="ExternalInput").ap()
    wk_d = nc.dram_tensor("wk", [ENC, HID], F32, kind="ExternalInput").ap()
    wv_d = nc.dram_tensor("wv", [ENC, HID], F32, kind="ExternalInput").ap()
    wo_d = nc.dram_tensor("wo", [HID, HID], F32, kind="ExternalInput").ap()
    bias_d = nc.dram_tensor("biases", [4, HID], F32, kind="ExternalInput").ap()

    out_d = nc.dram_tensor("out", [H, W, HID], F32, kind="ExternalOutput").ap()
    attn_d = nc.dram_tensor("attn", [H, HID, HID], F32, kind="ExternalOutput").ap()

    with tile.TileContext(nc) as tc, ExitStack() as ctx:
        singles = ctx.enter_context(tc.tile_pool(name="singles", bufs=1))
        io = ctx.enter_context(tc.tile_pool(name="io", bufs=3))
        work = ctx.enter_context(tc.tile_pool(name="work", bufs=2))
        epool = ctx.enter_context(tc.tile_pool(name="epool", bufs=3))
        small = ctx.enter_context(tc.tile_pool(name="small", bufs=4))
        stats = ctx.enter_context(tc.tile_pool(name="stats", bufs=4))
        ps_t = ctx.enter_context(tc.tile_pool(name="ps_t", bufs=2, space="PSUM"))
        ps_big = ctx.enter_context(tc.tile_pool(name="ps_big", bufs=3, space="PSUM"))
        ps_a = ctx.enter_context(tc.tile_pool(name="ps_a", bufs=2, space="PSUM"))
        ps_s = ctx.enter_context(tc.tile_pool(name="ps_s", bufs=1, space="PSUM"))

        # ---- constants / weights resident in SBUF ----
        ident = singles.tile([P, P], F32)
        make_identity(nc, ident)
        ones_cf = singles.tile([P, 1], F32)
        nc.vector.memset(ones_cf, 1.0)
        ones_col = singles.tile([P, 1], MMDT)
        nc.scalar.copy(out=ones_col, in_=ones_cf)
        ones_rf = singles.tile([1, P], F32)
        nc.vector.memset(ones_rf, 1.0)
        ones_row = singles.tile([1, P], MMDT)
        nc.scalar.copy(out=ones_row, in_=ones_rf)
        zeros_t = singles.tile([P, 64], F32)
        nc.vector.memset(zeros_t, 0.0)
        eps_t = singles.tile([P, 1], F32)
        nc.vector.memset(eps_t, EPS)

        wq_sb = singles.tile([P, HID_C, HID], MMDT)
        nc.sync.dma_start(wq_sb, _dview(wq_d.rearrange("(o p) n -> p o n", p=P)))
        wk_sb = singles.tile([P, ENC_C, HID], MMDT)
        nc.sync.dma_start(wk_sb, _dview(wk_d.rearrange("(o p) n -> p o n", p=P)))
        wv_sb = singles.tile([P, ENC_C, HID], MMDT)
        nc.sync.dma_start(wv_sb, _dview(wv_d.rearrange("(o p) n -> p o n", p=P)))
        wo_sb = singles.tile([P, HID_C, HID], MMDT)
        nc.sync.dma_start(wo_sb, _dview(wo_d.rearrange("(o p) n -> p o n", p=P)))
        bias_sb = singles.tile([1, 4, HID], MMDT)
        nc.sync.dma_start(bias_sb, _dview(bias_d[None, :, :]))

        def load_pair(dram, h0, d, tag):
            """Load dram[h0] -> rows 0-47, dram[h0+1] -> rows 64-111."""
            t = io.tile([PW, d], F32, tag=tag)
            nc.gpsimd.memset(t[32:64], 0.0)
            nc.sync.dma_start(t[0:48], dram[h0].rearrange("w d -> w d"))
            nc.sync.dma_start(t[64:112], dram[h0 + 1].rearrange("w d -> w d"))
            return t

        def layernorm(x_t, d, tag):
            """x_t: [PW, d] raw input -> returns normalized [PW, d] tile."""
            if d <= 512:
                st = stats.tile([PW, 6], F32, tag=f"st_{tag}")
                nc.vector.bn_stats(out=st, in_=x_t)
            else:
                gs = math.gcd(512, d)
                ng = d // gs
                st = stats.tile([PW, ng, 6], F32, tag=f"st_{tag}")
                for i in range(ng):
                    nc.vector.bn_stats(out=st[:, i, :], in_=x_t[:, ts(i, gs)])
            mv = stats.tile([PW, 2], F32, tag=f"mv_{tag}")
            nc.vector.bn_aggr(out=mv, in_=st)
            # rstd = 1/sqrt(var+eps) = exp(-0.5*ln(var+eps)); Ln+Exp share one
            # ACT table set (natural_log_exp_and_others) so no table thrash.
            lnv = stats.tile([PW, 1], F32, tag=f"lnv_{tag}")
            nc.scalar.activation(
                out=lnv, in_=mv[:, 1:2], func=mybir.ActivationFunctionType.Ln,
                bias=eps_t[:PW], scale=1.0,
            )
            rstd = stats.tile([PW, 1], F32, tag=f"rstd_{tag}")
            nc.scalar.activation(
                out=rstd, in_=lnv, func=mybir.ActivationFunctionType.Exp,
                scale=-0.5,
            )
            x_ln = work.tile([PW, d], F32, tag=f"ln_{tag}")
            nc.vector.tensor_scalar(
                out=x_ln, in0=x_t, scalar1=mv[:, 0:1], scalar2=rstd,
                op0=mybir.AluOpType.subtract, op1=mybir.AluOpType.mult,
            )
            return x_ln

        def transpose_pair(x_ln, nchunk, tag, dt=None):
            """[PW, 128*nchunk] -> [128, nchunk, PW] via PE transposes.

            Up to 4 transposed chunks share one PSUM bank (4*112*4B < 2KB)
            and are evacuated with a single copy.
            """
            xT = work.tile([P, nchunk, PW], dt or MMDT, tag=f"T_{tag}")
            for g0 in range(0, nchunk, 4):
                gn = min(4, nchunk - g0)
                pt = ps_t.tile([P, 4, PW], F32, tag="pst")
                for i in range(gn):
                    nc.tensor.transpose(
                        pt[:, i, :], x_ln[:, ts(g0 + i, P)], ident[:PW, :PW]
                    )
                nc.scalar.copy(
                    out=xT[:, g0:g0 + gn, :], in_=pt[:, :gn, :]
                )
            return xT

        def project(xT, w_sb, nchunk, bias_idx, tag):
            """pair projection: [PW,512] tile = xT.T @ W + bias."""
            pq = ps_big.tile([PW, HID], F32, tag="ps_big")
            for m in range(nchunk):
                nc.tensor.matmul(
                    pq, xT[:, m, :], w_sb[:, m, :],
                    start=(m == 0), stop=False,
                )
            # bias add: [1,PW] ones lhsT x [1,512] bias rhs
            nc.tensor.matmul(
                pq, ones_row[:, :PW], bias_sb[:, bias_idx, :],
                start=False, stop=True,
            )
            o = work.tile([PW, HID], MMDT, tag=f"proj_{tag}")
            nc.vector.tensor_copy(out=o, in_=pq)
            return o

        for j in range(H // 2):
            h0 = 2 * j
            x_t = load_pair(hid_d, h0, HID, "x_t")
            e_t = load_pair(enc_d, h0, ENC, "e_t")

            x_ln = layernorm(x_t, HID, "x")
            e_ln = layernorm(e_t, ENC, "e")
            xT = transpose_pair(x_ln, HID_C, "x")
            eT = transpose_pair(e_ln, ENC_C, "e")

            q_sb = project(xT, wq_sb, HID_C, 0, "q")
            k_sb = project(eT, wk_sb, ENC_C, 1, "k")
            v_sb = project(eT, wv_sb, ENC_C, 2, "v")
            vT = transpose_pair(v_sb.bitcast(F32), HID_C, "v")

            uT = work.tile([P, HID_C, PW], MMDT, tag="T_u")
            nc.scalar.copy(
                out=uT[:, :, 48:64],
                in_=zeros_t.rearrange("p (a b) -> p a b", a=HID_C),
            )
            for s in range(2):
                h = h0 + s
                sb = 64 * s
                E_sb = epool.tile([P, HID_C, HID], MMDT, tag="E")
                for m in range(HID_C):
                    pa = ps_a.tile([P, HID], F32, tag="ps_attn")
                    nc.tensor.matmul(
                        pa, q_sb[ds(sb, 48), ts(m, P)],
                        k_sb[ds(sb, 48), :],
                        start=True, stop=True,
                    )
                    nc.scalar.activation(
                        out=E_sb[:, m, :], in_=pa,
                        func=mybir.ActivationFunctionType.Exp,
                    )
                # column sums (over the partition/c axis) via ones-matmul
                pss = ps_s.tile([1, HID], F32, tag="ps_s")
                for m in range(HID_C):
                    nc.tensor.matmul(
                        pss, ones_col, E_sb[:, m, :],
                        start=(m == 0), stop=(m == HID_C - 1),
                    )
                s_sb = small.tile([1, HID], MMDT, tag="s_sb")
                nc.vector.tensor_copy(out=s_sb, in_=pss)
                # broadcast sums to 128 partitions, then reciprocal
                pr = ps_a.tile([P, HID], F32, tag="ps_attn")
                nc.tensor.matmul(
                    pr, ones_row[:1, :], s_sb, start=True, stop=True
                )
                r_sb = epool.tile([P, HID], F32, tag="r_sb")
                nc.vector.reciprocal_approx_fast(out=r_sb, in_=pr)
                # normalize E in place (softmax denominator)
                for m in range(HID_C):
                    nc.any.tensor_tensor(
                        out=E_sb[:, m, :], in0=E_sb[:, m, :], in1=r_sb,
                        op=mybir.AluOpType.mult,
                    )
                # attn output to DRAM
                nc.sync.dma_start(
                    _dview(attn_d[h].rearrange("(o p) n -> p o n", p=P)), E_sb
                )
                # u = v @ attn   [48, 512]
                pu = ps_big.tile([PW, HID], F32, tag="ps_big")
                for m in range(HID_C):
                    nc.tensor.matmul(
                        pu[:48], vT[:, m, ds(sb, 48)], E_sb[:, m, :],
                        start=(m == 0), stop=(m == HID_C - 1),
                    )
                u_s = work.tile([48, HID], F32, tag=f"u_s{s}")
                nc.vector.tensor_copy(out=u_s, in_=pu[:48])
                ptu = ps_t.tile([P, 4, PW], F32, tag="pst")
                for m in range(HID_C):
                    nc.tensor.transpose(
                        ptu[:, m, :48], u_s[:, ts(m, P)], ident[:48, :48]
                    )
                nc.scalar.copy(
                    out=uT[:, :, ds(sb, 48)], in_=ptu[:, :, :48]
                )

            po = ps_big.tile([PW, HID], F32, tag="ps_big")
            for m in range(HID_C):
                nc.tensor.matmul(
                    po, uT[:, m, :], wo_sb[:, m, :],
                    start=(m == 0), stop=False,
                )
            nc.tensor.matmul(
                po, ones_row[:, :PW], bias_sb[:, 3, :],
                start=False, stop=True,
            )
            o_sb = work.tile([PW, HID], F32, tag="o_sb")
            nc.any.tensor_tensor(
                out=o_sb, in0=po, in1=x_t, op=mybir.AluOpType.add
            )
            nc.sync.dma_start(out_d[h0], o_sb[0:48])
            nc.sync.dma_start(out_d[h0 + 1], o_sb[64:112])

    nc.compile()
    return nc


def kernel(**inputs):
    global LAST_RESULTS
    hidden = np.asarray(inputs["hidden"], dtype=np.float32)
    enc = np.asarray(inputs["enc"], dtype=np.float32)
    ln_g = np.asarray(inputs["ln_g"], dtype=np.float32)
    ln_b = np.asarray(inputs["ln_b"], dtype=np.float32)
    eln_g = np.asarray(inputs["eln_g"], dtype=np.float32)
    eln_b = np.asarray(inputs["eln_b"], dtype=np.float32)
    Wq = np.asarray(inputs["Wq"], dtype=np.float32)
    bq = np.asarray(inputs["bq"], dtype=np.float32)
    Wk = np.asarray(inputs["Wk"], dtype=np.float32)
    bk = np.asarray(inputs["bk"], dtype=np.float32)
    Wv = np.asarray(inputs["Wv"], dtype=np.float32)
    bv = np.asarray(inputs["bv"], dtype=np.float32)
    Wo = np.asarray(inputs["Wo"], dtype=np.float32)
    bo = np.asarray(inputs["bo"], dtype=np.float32)

    scale = math.sqrt(1.0 / W)
    # fold LN affine + attn scale into the projections (exact for g=1, b=0)
    wq_eff = (ln_g[:, None] * Wq) * scale
    bq_eff = (ln_b @ Wq + bq) * scale
    wk_eff = eln_g[:, None] * Wk
    bk_eff = eln_b @ Wk + bk
    wv_eff = eln_g[:, None] * Wv
    bv_eff = eln_b @ Wv + bv
    biases = np.stack([bq_eff, bk_eff, bv_eff, bo]).astype(np.float32)

    nc = build_program()

    in_maps = []
    for b in range(N_CORES):
        in_maps.append({
            "hidden": np.ascontiguousarray(hidden[b]),
            "enc": np.ascontiguousarray(enc[b]),
            "wq": np.ascontiguousarray(wq_eff),
            "wk": np.ascontiguousarray(wk_eff),
            "wv": np.ascontiguousarray(wv_eff),
            "wo": np.ascontiguousarray(Wo),
            "biases": biases,
        })

    res = run_bass_kernel_spmd(
        nc, in_maps, core_ids=list(range(N_CORES)),
        trace=bool(os.environ.get("KERNEL_TRACE")),
    )
    LAST_RESULTS = res

    out = np.stack([res.results[b]["out"] for b in range(N_CORES)])
    attn = np.stack([res.results[b]["attn"] for b in range(N_CORES)])
    return out, attn


# revision 18
# speedup vs baseline: 1.7464x; 1.1925x over previous
"""Trainium2 Bass kernel for nn_LionCrossAttentionDimH.

Computes, per (b, h) slice (H treated as independent "heads"):
    x = LN(hidden); e = LN(enc)
    q = x@Wq + bq ; k = e@Wk + bk ; v = e@Wv + bv        [48, 512]
    attn = softmax((q^T k) * sqrt(1/48), axis=c)          [512, 512]
    out  = (v @ attn) @ Wo + bo + hidden                  [48, 512]
Returns (out, attn) with full shapes [8,48,48,512] and [8,48,512,512].

Sharding: data-parallel over batch B=8 -> 8 NeuronCores (SPMD, no
collectives). Weights replicated; LN gains and the attn scale are folded
into the projection weights on the host.

Layout: h-slices are processed in pairs. Pair tensors use 112 partition
rows with slice0 at rows 0-47 and slice1 at rows 64-111 (pad rows 48-63
zeroed) so every per-slice engine access starts at partition 0 or 64 —
hardware requires 32-aligned partition starts.

Softmax: attn kept natural ([c,k], c on partitions). exp on ScalarE.
Column sums are folded into the v@attn matmul via an extra ones-column in
the vT operand (psum row 64 collects the sums). 1/sum via the fast DVE
reciprocal; u^T is rescaled per-partition during PSUM evacuation, while
the attn output tile is normalized with tensor_tensor ops off the
critical path.
"""

import math
import os
import sys
from contextlib import ExitStack

for _p in ("/opt/trn_rl_repo",):
    if _p not in sys.path:
        sys.path.insert(0, _p)

import numpy as np

import concourse.bass as bass
import concourse.tile as tile
from concourse import bacc, mybir
from concourse.bass import ds, ts
from concourse.bass_utils import run_bass_kernel_spmd
from concourse.masks import make_identity

# ---- problem constants (hardcoded per contract) ----
B, H, W = 8, 48, 48
HID, ENC = 512, 768
N_CORES = 8
P = 128
HID_C = HID // P   # 4
ENC_C = ENC // P   # 6
EPS = 1e-5
PW = 112           # padded pair width (rows 0-47 slice0, 64-111 slice1)

F32 = mybir.dt.float32
F32R = mybir.dt.float32r
I32 = mybir.dt.int32
BF16 = mybir.dt.bfloat16

MM_MODE = os.environ.get("KERNEL_MM", "f32r")
MMDT = F32R if MM_MODE == "f32r" else F32
WARM_EVERY = int(os.environ.get("KERNEL_WARM", "1"))  # emit bf16 warm-up MMs

LAST_RESULTS = None  # BassKernelResults of the most recent kernel() call


def _dview(ap):
    """Bitcast a DRAM-side fp32 AP to the matmul dtype (same bytes)."""
    if MM_MODE == "f32r":
        return ap.bitcast(F32R)
    return ap


def build_program(use_bias=False):
    nc = bacc.Bacc(
        "TRN2", target_bir_lowering=False, debug=False, num_devices=N_CORES
    )

    hid_d = nc.dram_tensor("hidden", [H, W, HID], F32, kind="ExternalInput").ap()
    enc_d = nc.dram_tensor("enc", [H, W, ENC], F32, kind="ExternalInput").ap()
    wq_d = nc.dram_tensor("wq", [HID, HID], F32, kind="ExternalInput").ap()
    wk_d = nc.dram_tensor("wk", [ENC, HID], F32, kind="ExternalInput").ap()
    wv_d = nc.dram_tensor("wv", [ENC, HID], F32, kind="ExternalInput").ap()
    wo_d = nc.dram_tensor("wo", [HID, HID], F32, kind="ExternalInput").ap()
    bias_d = nc.dram_tensor("biases", [4, HID], F32, kind="ExternalInput").ap()

    out_d = nc.dram_tensor("out", [H, W, HID], F32, kind="ExternalOutput").ap()
    attn_d = nc.dram_tensor("attn", [H, HID, HID], F32, kind="ExternalOutput").ap()

    with tile.TileContext(nc) as tc, ExitStack() as ctx:
        singles = ctx.enter_context(tc.tile_pool(name="singles", bufs=1))
        io = ctx.enter_context(tc.tile_pool(name="io", bufs=3))
        work = ctx.enter_context(tc.tile_pool(name="work", bufs=2))
        epool = ctx.enter_context(tc.tile_pool(name="epool", bufs=2))
        small = ctx.enter_context(tc.tile_pool(name="small", bufs=4))
        stats = ctx.enter_context(tc.tile_pool(name="stats", bufs=4))
        ps_t = ctx.enter_context(tc.tile_pool(name="ps_t", bufs=2, space="PSUM"))
        ps_big = ctx.enter_context(tc.tile_pool(name="ps_big", bufs=3, space="PSUM"))
        ps_a = ctx.enter_context(tc.tile_pool(name="ps_a", bufs=2, space="PSUM"))
        ps_w = ctx.enter_context(tc.tile_pool(name="ps_w", bufs=1, space="PSUM"))

        # ---- constants / weights resident in SBUF ----
        ident = singles.tile([P, P], F32)
        make_identity(nc, ident)
        ones_rf = singles.tile([1, P], F32)
        nc.vector.memset(ones_rf, 1.0)
        ones_row = singles.tile([1, P], MMDT)
        nc.scalar.copy(out=ones_row, in_=ones_rf)
        ones_cf = singles.tile([P, 4], F32)
        nc.vector.memset(ones_cf, 1.0)
        zeros_t = singles.tile([P, 68], F32)
        nc.vector.memset(zeros_t, 0.0)
        # bf16 tiles for HAM warm-up matmuls (bf16 MMs count as PE activity
        # for the clock gate; fp32/f32r modes do not, leaving PE at 1.2 GHz)
        wa = singles.tile([P, 8], BF16)
        nc.vector.memset(wa, 1.0)
        wb = singles.tile([P, 64], BF16)
        nc.vector.memset(wb, 1.0)
        warm_ps = ps_w.tile([8, 64], F32, tag="warm")

        def warm():
            if WARM_EVERY:
                nc.tensor.matmul(warm_ps, wa, wb, start=True, stop=True)

        wq_sb = singles.tile([P, HID_C, HID], MMDT)
        nc.sync.dma_start(wq_sb, _dview(wq_d.rearrange("(o p) n -> p o n", p=P)))
        wk_sb = singles.tile([P, ENC_C, HID], MMDT)
        nc.sync.dma_start(wk_sb, _dview(wk_d.rearrange("(o p) n -> p o n", p=P)))
        wv_sb = singles.tile([P, ENC_C, HID], MMDT)
        nc.sync.dma_start(wv_sb, _dview(wv_d.rearrange("(o p) n -> p o n", p=P)))
        wo_sb = singles.tile([P, HID_C, HID], MMDT)
        nc.sync.dma_start(wo_sb, _dview(wo_d.rearrange("(o p) n -> p o n", p=P)))
        bias_sb = singles.tile([1, 4, HID], MMDT)
        nc.sync.dma_start(bias_sb, _dview(bias_d[None, :, :]))

        def load_pair(dram, h0, d, tag):
            """Load dram[h0] -> rows 0-47, dram[h0+1] -> rows 64-111."""
            t = io.tile([PW, d], F32, tag=tag)
            nc.vector.memset(t[32:64], 0.0)
            nc.sync.dma_start(t[0:48], dram[h0])
            nc.sync.dma_start(t[64:112], dram[h0 + 1])
            return t

        def rsqrt_dve(va, tag):
            """1/sqrt(va) entirely on VectorE (bit-trick seed + 2 NR steps).

            Avoids Sqrt/Ln on ScalarE so the only ACT table set ever loaded
            is the Exp one (a table swap costs ~2.7us and was thrashing).
            """
            # seed = 0x5F3759DF - (bits >> 1), built from same-class ALU ops:
            # (bits>>1) ^ 0xFFFFFFFF == -(bits>>1) - 1, then add magic+1.
            sh = stats.tile([PW, 1], I32, tag=f"sh_{tag}")
            nc.vector.tensor_scalar(
                out=sh, in0=va.bitcast(I32), scalar1=1, scalar2=-1,
                op0=mybir.AluOpType.logical_shift_right,
                op1=mybir.AluOpType.bitwise_xor,
            )
            y = stats.tile([PW, 1], I32, tag=f"seed_{tag}")
            nc.vector.tensor_scalar(
                out=y, in0=sh, scalar1=0x5F3759DF + 1, scalar2=None,
                op0=mybir.AluOpType.add,
            )
            y = y.bitcast(F32)
            for it in range(2):
                t1 = stats.tile([PW, 1], F32, tag=f"nr1_{tag}{it}")
                nc.vector.tensor_tensor(
                    out=t1, in0=va, in1=y, op=mybir.AluOpType.mult
                )
                nc.vector.tensor_tensor(
                    out=t1, in0=t1, in1=y, op=mybir.AluOpType.mult
                )
                nc.vector.tensor_scalar(
                    out=t1, in0=t1, scalar1=-0.5, scalar2=1.5,
                    op0=mybir.AluOpType.mult, op1=mybir.AluOpType.add,
                )
                y2 = stats.tile([PW, 1], F32, tag=f"nr2_{tag}{it}")
                nc.vector.tensor_tensor(
                    out=y2, in0=y, in1=t1, op=mybir.AluOpType.mult
                )
                y = y2
            return y

        def layernorm(x_t, d, tag):
            """x_t: [PW, d] raw input -> returns normalized [PW, d] tile."""
            if d <= 512:
                st = stats.tile([PW, 6], F32, tag=f"st_{tag}")
                nc.vector.bn_stats(out=st, in_=x_t)
            else:
                gs = math.gcd(512, d)
                ng = d // gs
                st = stats.tile([PW, ng, 6], F32, tag=f"st_{tag}")
                for i in range(ng):
                    nc.vector.bn_stats(out=st[:, i, :], in_=x_t[:, ts(i, gs)])
            mv = stats.tile([PW, 2], F32, tag=f"mv_{tag}")
            nc.vector.bn_aggr(out=mv, in_=st)
            va = stats.tile([PW, 1], F32, tag=f"va_{tag}")
            nc.vector.tensor_scalar(
                out=va, in0=mv[:, 1:2], scalar1=EPS, scalar2=None,
                op0=mybir.AluOpType.add,
            )
            rstd = rsqrt_dve(va, tag)
            x_ln = work.tile([PW, d], F32, tag=f"ln_{tag}")
            nc.vector.tensor_scalar(
                out=x_ln, in0=x_t, scalar1=mv[:, 0:1], scalar2=rstd,
                op0=mybir.AluOpType.subtract, op1=mybir.AluOpType.mult,
            )
            return x_ln

        def transpose_pair(x_ln, nchunk, tag, dt=None):
            """[PW, 128*nchunk] -> [128, nchunk, PW] via PE transposes.

            Up to 4 transposed chunks share one PSUM bank (4*112*4B < 2KB)
            and are evacuated with a single copy.
            """
            xT = work.tile([P, nchunk, PW], dt or MMDT, tag=f"T_{tag}")
            for g0 in range(0, nchunk, 4):
                gn = min(4, nchunk - g0)
                pt = ps_t.tile([P, 4, PW], F32, tag="pst")
                for i in range(gn):
                    nc.tensor.transpose(
                        pt[:, i, :], x_ln[:, ts(g0 + i, P)], ident[:PW, :PW]
                    )
                nc.scalar.copy(
                    out=xT[:, g0:g0 + gn, :], in_=pt[:, :gn, :]
                )
            return xT

        def project(xT, w_sb, nchunk, bias_idx, tag):
            """pair projection: [PW,512] tile = xT.T @ W (+ bias)."""
            pq = ps_big.tile([PW, HID], F32, tag="ps_big")
            for m in range(nchunk):
                nc.tensor.matmul(
                    pq, xT[:, m, :], w_sb[:, m, :],
                    start=(m == 0), stop=(m == nchunk - 1) and not use_bias,
                )
            if use_bias:
                nc.tensor.matmul(
                    pq, ones_row[:, :PW], bias_sb[:, bias_idx, :],
                    start=False, stop=True,
                )
            o = work.tile([PW, HID], MMDT, tag=f"proj_{tag}")
            nc.vector.tensor_copy(out=o, in_=pq)
            return o

        for j in range(H // 2):
            h0 = 2 * j
            x_t = load_pair(hid_d, h0, HID, "x_t")
            e_t = load_pair(enc_d, h0, ENC, "e_t")

            x_ln = layernorm(x_t, HID, "x")
            e_ln = layernorm(e_t, ENC, "e")
            xT = transpose_pair(x_ln, HID_C, "x")
            warm()
            eT = transpose_pair(e_ln, ENC_C, "e")

            q_sb = project(xT, wq_sb, HID_C, 0, "q")
            warm()
            k_sb = project(eT, wk_sb, ENC_C, 1, "k")
            warm()
            v_sb = project(eT, wv_sb, ENC_C, 2, "v")

            # vT layout [128, HID_C, 130]:
            #   cols 0-47  slice0 v^T   48-63 zeros  64  ones
            #   cols 65-112 slice1 v^T  113-127 zeros 128 ones  129 zero
            # The ones column folds the attn column-sum into the v@attn
            # matmul (psum row 64 = sum over c of attn chunk).
            vT = work.tile([P, HID_C, 130], MMDT, tag="T_v")
            for g0 in range(0, HID_C, 4):
                pt = ps_t.tile([P, 4, PW], F32, tag="pst")
                for i in range(4):
                    nc.tensor.transpose(
                        pt[:, i, :], v_sb.bitcast(F32)[:, ts(g0 + i, P)],
                        ident[:PW, :PW],
                    )
                nc.scalar.copy(out=vT[:, :, 0:64], in_=pt[:, :, 0:64])
                nc.scalar.copy(out=vT[:, :, 65:113], in_=pt[:, :, 64:112])
            nc.scalar.copy(
                out=vT[:, :, 64:65],
                in_=ones_cf.rearrange("p (a b) -> p a b", a=4),
            )
            nc.scalar.copy(
                out=vT[:, :, 129:130],
                in_=ones_cf.rearrange("p (a b) -> p a b", a=4),
            )
            nc.scalar.copy(
                out=vT[:, :, 113:129],
                in_=zeros_t[:, :64].rearrange("p (a b) -> p a b", a=4),
            )

            # attn logits + exp for both slices, interleaved so the two
            # K=48 matmuls land in disjoint PE row groups and overlap.
            E_pair = [
                epool.tile([P, HID_C, HID], MMDT, tag="E0", name="E0"),
                epool.tile([P, HID_C, HID], MMDT, tag="E1", name="E1"),
            ]
            for m in range(HID_C):
                for s in range(2):
                    sb = 64 * s
                    pa = ps_a.tile([P, HID], F32, tag="ps_attn")
                    nc.tensor.matmul(
                        pa, q_sb[ds(sb, 48), ts(m, P)],
                        k_sb[ds(sb, 48), :],
                        start=True, stop=True,
                    )
                    nc.scalar.activation(
                        out=E_pair[s][:, m, :], in_=pa,
                        func=mybir.ActivationFunctionType.Exp,
                    )
                warm()

            uT = work.tile([P, HID_C, PW], MMDT, tag="T_u")
            nc.scalar.copy(
                out=uT[:, :, 48:64],
                in_=zeros_t[:, :64].rearrange("p (a b) -> p a b", a=4),
            )
            for s in range(2):
                h = h0 + s
                sb = 64 * s
                E_sb = E_pair[s]
                # u_unnorm = v @ exp(A); psum row 64 = column sums of exp(A)
                pu = ps_big.tile([PW, HID], F32, tag="ps_big")
                for m in range(HID_C):
                    nc.tensor.matmul(
                        pu[:65], vT[:, m, ds(65 * s, 65)], E_sb[:, m, :],
                        start=(m == 0), stop=(m == HID_C - 1),
                    )
                warm()
                s_sb = small.tile([1, HID], MMDT, tag="s_sb")
                nc.vector.tensor_copy(out=s_sb, in_=pu[64:65])
                # r = 1/s broadcast to all partitions (for attn normalize)
                pr = ps_a.tile([P, HID], F32, tag="ps_attn")
                nc.tensor.matmul(
                    pr, ones_row[:1, :], s_sb, start=True, stop=True
                )
                r_sb = epool.tile([P, HID], F32, tag="r_sb")
                nc.vector.reciprocal_approx_fast(out=r_sb, in_=pr)
                # rT = 1/s with k on partitions (for u^T rescale): transpose
                # s — four [1,128] -> [128,1] PE transposes into one bank.
                pst_s = ps_t.tile([P, 4], F32, tag="pst")
                for m in range(HID_C):
                    nc.tensor.transpose(
                        pst_s[:, m:m + 1],
                        s_sb.bitcast(F32)[:, ts(m, P)], ident[:1, :1],
                    )
                rT = small.tile([P, 4], F32, tag="rT")
                nc.vector.reciprocal_approx_fast(out=rT, in_=pst_s)
                # normalize attn output (off the out2 critical path)
                for m in range(HID_C):
                    nc.any.tensor_tensor(
                        out=E_sb[:, m, :], in0=E_sb[:, m, :], in1=r_sb,
                        op=mybir.AluOpType.mult,
                    )
                nc.sync.dma_start(
                    _dview(attn_d[h].rearrange("(o p) n -> p o n", p=P)), E_sb
                )
                # u^T with per-partition 1/s rescale during PSUM evacuation
                u_s = work.tile([48, HID], F32, tag=f"u_s{s}")
                nc.vector.tensor_copy(out=u_s, in_=pu[:48])
                ptu = ps_t.tile([P, 4, PW], F32, tag="pst")
                for m in range(HID_C):
                    nc.tensor.transpose(
                        ptu[:, m, :48], u_s[:, ts(m, P)], ident[:48, :48]
                    )
                for m in range(HID_C):
                    nc.vector.tensor_scalar(
                        out=uT[:, m, ds(sb, 48)], in0=ptu[:, m, :48],
                        scalar1=rT[:, m:m + 1], scalar2=None,
                        op0=mybir.AluOpType.mult,
                    )

            po = ps_big.tile([PW, HID], F32, tag="ps_big")
            for m in range(HID_C):
                nc.tensor.matmul(
                    po, uT[:, m, :], wo_sb[:, m, :],
                    start=(m == 0), stop=(m == HID_C - 1) and not use_bias,
                )
            if use_bias:
                nc.tensor.matmul(
                    po, ones_row[:, :PW], bias_sb[:, 3, :],
                    start=False, stop=True,
                )
            warm()
            o_sb = work.tile([PW, HID], F32, tag="o_sb")
            nc.any.tensor_tensor(
                out=o_sb, in0=po, in1=x_t, op=mybir.AluOpType.add
            )
            nc.sync.dma_start(out_d[h0], o_sb[0:48])
            nc.sync.dma_start(out_d[h0 + 1], o_sb[64:112])

    nc.compile()
    return nc


def kernel(**inputs):
    global LAST_RESULTS
    hidden = np.asarray(inputs["hidden"], dtype=np.float32)
    enc = np.asarray(inputs["enc"], dtype=np.float32)
    ln_g = np.asarray(inputs["ln_g"], dtype=np.float32)
    ln_b = np.asarray(inputs["ln_b"], dtype=np.float32)
    eln_g = np.asarray(inputs["eln_g"], dtype=np.float32)
    eln_b = np.asarray(inputs["eln_b"], dtype=np.float32)
    Wq = np.asarray(inputs["Wq"], dtype=np.float32)
    bq = np.asarray(inputs["bq"], dtype=np.float32)
    Wk = np.asarray(inputs["Wk"], dtype=np.float32)
    bk = np.asarray(inputs["bk"], dtype=np.float32)
    Wv = np.asarray(inputs["Wv"], dtype=np.float32)
    bv = np.asarray(inputs["bv"], dtype=np.float32)
    Wo = np.asarray(inputs["Wo"], dtype=np.float32)
    bo = np.asarray(inputs["bo"], dtype=np.float32)

    scale = math.sqrt(1.0 / W)
    # fold LN affine + attn scale into the projections (exact for g=1, b=0)
    wq_eff = (ln_g[:, None] * Wq) * scale
    bq_eff = (ln_b @ Wq + bq) * scale
    wk_eff = eln_g[:, None] * Wk
    bk_eff = eln_b @ Wk + bk
    wv_eff = eln_g[:, None] * Wv
    bv_eff = eln_b @ Wv + bv
    biases = np.stack([bq_eff, bk_eff, bv_eff, bo]).astype(np.float32)

    nc = build_program(use_bias=bool(np.any(biases)))

    in_maps = []
    for b in range(N_CORES):
        in_maps.append({
            "hidden": np.ascontiguousarray(hidden[b]),
            "enc": np.ascontiguousarray(enc[b]),
            "wq": np.ascontiguousarray(wq_eff),
            "wk": np.ascontiguousarray(wk_eff),
            "wv": np.ascontiguousarray(wv_eff),
            "wo": np.ascontiguousarray(Wo),
            "biases": biases,
        })

    res = run_bass_kernel_spmd(
        nc, in_maps, core_ids=list(range(N_CORES)),
        trace=bool(os.environ.get("KERNEL_TRACE")),
    )
    LAST_RESULTS = res

    out = np.stack([res.results[b]["out"] for b in range(N_CORES)])
    attn = np.stack([res.results[b]["attn"] for b in range(N_CORES)])
    return out, attn


# revision 22
# speedup vs baseline: 1.7477x; 1.0008x over previous
"""Trainium2 Bass kernel for nn_LionCrossAttentionDimH.

Computes, per (b, h) slice (H treated as independent "heads"):
    x = LN(hidden); e = LN(enc)
    q = x@Wq + bq ; k = e@Wk + bk ; v = e@Wv + bv        [48, 512]
    attn = softmax((q^T k) * sqrt(1/48), axis=c)          [512, 512]
    out  = (v @ attn) @ Wo + bo + hidden                  [48, 512]
Returns (out, attn) with full shapes [8,48,48,512] and [8,48,512,512].

Sharding: data-parallel over batch B=8 -> 8 NeuronCores (SPMD, no
collectives). Weights replicated; LN gains and the attn scale are folded
into the projection weights on the host.

Layout: h-slices are processed in pairs. Pair tensors use 112 partition
rows with slice0 at rows 0-47 and slice1 at rows 64-111 (pad rows 48-63
zeroed) so every per-slice engine access starts at partition 0 or 64 —
hardware requires 32-aligned partition starts.

Softmax: attn kept natural ([c,k], c on partitions). exp on ScalarE.
Column sums are folded into the v@attn matmul via an extra ones-column in
the vT operand (psum row 64 collects the sums). 1/sum via the fast DVE
reciprocal; u^T is rescaled per-partition during PSUM evacuation, while
the attn output tile is normalized with tensor_tensor ops off the
critical path.
"""

import math
import os
import sys
from contextlib import ExitStack

for _p in ("/opt/trn_rl_repo",):
    if _p not in sys.path:
        sys.path.insert(0, _p)

import numpy as np

import concourse.bass as bass
import concourse.tile as tile
from concourse import bacc, mybir
from concourse.bass import ds, ts
from concourse.bass_utils import run_bass_kernel_spmd
from concourse.masks import make_identity

# ---- problem constants (hardcoded per contract) ----
B, H, W = 8, 48, 48
HID, ENC = 512, 768
N_CORES = 8
P = 128
HID_C = HID // P   # 4
ENC_C = ENC // P   # 6
EPS = 1e-5
PW = 112           # padded pair width (rows 0-47 slice0, 64-111 slice1)

F32 = mybir.dt.float32
F32R = mybir.dt.float32r
I32 = mybir.dt.int32
BF16 = mybir.dt.bfloat16

MM_MODE = os.environ.get("KERNEL_MM", "f32r")
MMDT = F32R if MM_MODE == "f32r" else F32
TDT = F32R if os.environ.get("KERNEL_TDT", "f32") == "f32r" else F32
WARM_EVERY = int(os.environ.get("KERNEL_WARM", "0"))  # emit bf16 warm-up MMs

LAST_RESULTS = None  # BassKernelResults of the most recent kernel() call


def _dview(ap):
    """Bitcast a DRAM-side fp32 AP to the matmul dtype (same bytes)."""
    if MM_MODE == "f32r":
        return ap.bitcast(F32R)
    return ap


def build_program(use_bias=False):
    nc = bacc.Bacc(
        "TRN2", target_bir_lowering=False, debug=False, num_devices=N_CORES
    )

    hid_d = nc.dram_tensor("hidden", [H, W, HID], F32, kind="ExternalInput").ap()
    enc_d = nc.dram_tensor("enc", [H, W, ENC], F32, kind="ExternalInput").ap()
    wq_d = nc.dram_tensor("wq", [HID, HID], F32, kind="ExternalInput").ap()
    wk_d = nc.dram_tensor("wk", [ENC, HID], F32, kind="ExternalInput").ap()
    wv_d = nc.dram_tensor("wv", [ENC, HID], F32, kind="ExternalInput").ap()
    wo_d = nc.dram_tensor("wo", [HID, HID], F32, kind="ExternalInput").ap()
    bias_d = nc.dram_tensor("biases", [4, HID], F32, kind="ExternalInput").ap()

    out_d = nc.dram_tensor("out", [H, W, HID], F32, kind="ExternalOutput").ap()
    attn_d = nc.dram_tensor("attn", [H, HID, HID], F32, kind="ExternalOutput").ap()

    with tile.TileContext(nc) as tc, ExitStack() as ctx:
        singles = ctx.enter_context(tc.tile_pool(name="singles", bufs=1))
        io = ctx.enter_context(tc.tile_pool(name="io", bufs=3))
        work = ctx.enter_context(tc.tile_pool(name="work", bufs=2))
        epool = ctx.enter_context(tc.tile_pool(name="epool", bufs=2))
        small = ctx.enter_context(tc.tile_pool(name="small", bufs=4))
        stats = ctx.enter_context(tc.tile_pool(name="stats", bufs=4))
        ps_t = ctx.enter_context(tc.tile_pool(name="ps_t", bufs=2, space="PSUM"))
        ps_big = ctx.enter_context(tc.tile_pool(name="ps_big", bufs=3, space="PSUM"))
        ps_a = ctx.enter_context(tc.tile_pool(name="ps_a", bufs=2, space="PSUM"))
        ps_w = ctx.enter_context(tc.tile_pool(name="ps_w", bufs=1, space="PSUM"))

        # ---- constants / weights resident in SBUF ----
        ident = singles.tile([P, P], F32)
        make_identity(nc, ident)
        ident_t = singles.tile([P, P], TDT)
        nc.scalar.copy(out=ident_t, in_=ident)
        ones_rf = singles.tile([1, P], F32)
        nc.vector.memset(ones_rf, 1.0)
        ones_row = singles.tile([1, P], MMDT)
        nc.scalar.copy(out=ones_row, in_=ones_rf)
        ones_cf = singles.tile([P, 4], F32)
        nc.vector.memset(ones_cf, 1.0)
        zeros_t = singles.tile([P, 68], F32)
        nc.vector.memset(zeros_t, 0.0)
        # bf16 tiles for HAM warm-up matmuls (bf16 MMs count as PE activity
        # for the clock gate; fp32/f32r modes do not, leaving PE at 1.2 GHz)
        wa = singles.tile([P, 8], BF16)
        nc.vector.memset(wa, 1.0)
        wb = singles.tile([P, 64], BF16)
        nc.vector.memset(wb, 1.0)
        warm_ps = ps_w.tile([8, 64], F32, tag="warm")

        def warm():
            if WARM_EVERY:
                nc.tensor.matmul(warm_ps, wa, wb, start=True, stop=True)

        wq_sb = singles.tile([P, HID_C, HID], MMDT)
        nc.sync.dma_start(wq_sb, _dview(wq_d.rearrange("(o p) n -> p o n", p=P)))
        wk_sb = singles.tile([P, ENC_C, HID], MMDT)
        nc.sync.dma_start(wk_sb, _dview(wk_d.rearrange("(o p) n -> p o n", p=P)))
        wv_sb = singles.tile([P, ENC_C, HID], MMDT)
        nc.sync.dma_start(wv_sb, _dview(wv_d.rearrange("(o p) n -> p o n", p=P)))
        wo_sb = singles.tile([P, HID_C, HID], MMDT)
        nc.sync.dma_start(wo_sb, _dview(wo_d.rearrange("(o p) n -> p o n", p=P)))
        bias_sb = singles.tile([1, 4, HID], MMDT)
        nc.sync.dma_start(bias_sb, _dview(bias_d[None, :, :]))

        def load_pair(dram, h0, d, tag):
            """Load dram[h0] -> rows 0-47, dram[h0+1] -> rows 64-111."""
            t = io.tile([PW, d], F32, tag=tag)
            nc.vector.memset(t[32:64], 0.0)
            nc.sync.dma_start(t[0:48], dram[h0])
            nc.sync.dma_start(t[64:112], dram[h0 + 1])
            return t

        def rsqrt_dve(va, tag):
            """1/sqrt(va) entirely on VectorE (bit-trick seed + 2 NR steps).

            Avoids Sqrt/Ln on ScalarE so the only ACT table set ever loaded
            is the Exp one (a table swap costs ~2.7us and was thrashing).
            """
            # seed = 0x5F3759DF - (bits >> 1), built from same-class ALU ops:
            # (bits>>1) ^ 0xFFFFFFFF == -(bits>>1) - 1, then add magic+1.
            sh = stats.tile([PW, 1], I32, tag=f"sh_{tag}")
            nc.vector.tensor_scalar(
                out=sh, in0=va.bitcast(I32), scalar1=1, scalar2=-1,
                op0=mybir.AluOpType.logical_shift_right,
                op1=mybir.AluOpType.bitwise_xor,
            )
            y = stats.tile([PW, 1], I32, tag=f"seed_{tag}")
            nc.vector.tensor_scalar(
                out=y, in0=sh, scalar1=0x5F3759DF + 1, scalar2=None,
                op0=mybir.AluOpType.add,
            )
            y = y.bitcast(F32)
            for it in range(2):
                t1 = stats.tile([PW, 1], F32, tag=f"nr1_{tag}{it}")
                nc.vector.tensor_tensor(
                    out=t1, in0=va, in1=y, op=mybir.AluOpType.mult
                )
                nc.vector.tensor_tensor(
                    out=t1, in0=t1, in1=y, op=mybir.AluOpType.mult
                )
                nc.vector.tensor_scalar(
                    out=t1, in0=t1, scalar1=-0.5, scalar2=1.5,
                    op0=mybir.AluOpType.mult, op1=mybir.AluOpType.add,
                )
                y2 = stats.tile([PW, 1], F32, tag=f"nr2_{tag}{it}")
                nc.vector.tensor_tensor(
                    out=y2, in0=y, in1=t1, op=mybir.AluOpType.mult
                )
                y = y2
            return y

        def layernorm(x_t, d, tag):
            """x_t: [PW, d] raw input -> returns normalized [PW, d] tile."""
            if d <= 512:
                st = stats.tile([PW, 6], F32, tag=f"st_{tag}")
                nc.vector.bn_stats(out=st, in_=x_t)
            else:
                gs = math.gcd(512, d)
                ng = d // gs
                st = stats.tile([PW, ng, 6], F32, tag=f"st_{tag}")
                for i in range(ng):
                    nc.vector.bn_stats(out=st[:, i, :], in_=x_t[:, ts(i, gs)])
            mv = stats.tile([PW, 2], F32, tag=f"mv_{tag}")
            nc.vector.bn_aggr(out=mv, in_=st)
            va = stats.tile([PW, 1], F32, tag=f"va_{tag}")
            nc.vector.tensor_scalar(
                out=va, in0=mv[:, 1:2], scalar1=EPS, scalar2=None,
                op0=mybir.AluOpType.add,
            )
            rstd = rsqrt_dve(va, tag)
            x_ln = work.tile([PW, d], TDT, tag=f"ln_{tag}")
            nc.vector.tensor_scalar(
                out=x_ln, in0=x_t, scalar1=mv[:, 0:1], scalar2=rstd,
                op0=mybir.AluOpType.subtract, op1=mybir.AluOpType.mult,
            )
            return x_ln

        def transpose_pair(x_ln, nchunk, tag, dt=None):
            """[PW, 128*nchunk] -> [128, nchunk, PW] via PE transposes.

            Up to 4 transposed chunks share one PSUM bank (4*112*4B < 2KB)
            and are evacuated with a single copy.
            """
            xT = work.tile([P, nchunk, PW], dt or MMDT, tag=f"T_{tag}")
            for g0 in range(0, nchunk, 4):
                gn = min(4, nchunk - g0)
                pt = ps_t.tile([P, 4, PW], TDT, tag="pst")
                for i in range(gn):
                    nc.tensor.transpose(
                        pt[:, i, :], x_ln[:, ts(g0 + i, P)], ident_t[:PW, :PW]
                    )
                nc.scalar.copy(
                    out=xT[:, g0:g0 + gn, :], in_=pt[:, :gn, :]
                )
            return xT

        def project(xT, w_sb, nchunk, bias_idx, tag):
            """pair projection: [PW,512] tile = xT.T @ W (+ bias)."""
            pq = ps_big.tile([PW, HID], F32, tag="ps_big")
            for m in range(nchunk):
                nc.tensor.matmul(
                    pq, xT[:, m, :], w_sb[:, m, :],
                    start=(m == 0), stop=(m == nchunk - 1) and not use_bias,
                )
            if use_bias:
                nc.tensor.matmul(
                    pq, ones_row[:, :PW], bias_sb[:, bias_idx, :],
                    start=False, stop=True,
                )
            o = work.tile([PW, HID], MMDT, tag=f"proj_{tag}")
            nc.vector.tensor_copy(out=o, in_=pq)
            return o

        for j in range(H // 2):
            h0 = 2 * j
            x_t = load_pair(hid_d, h0, HID, "x_t")
            e_t = load_pair(enc_d, h0, ENC, "e_t")

            x_ln = layernorm(x_t, HID, "x")
            e_ln = layernorm(e_t, ENC, "e")
            xT = transpose_pair(x_ln, HID_C, "x")
            warm()
            eT = transpose_pair(e_ln, ENC_C, "e")

            q_sb = project(xT, wq_sb, HID_C, 0, "q")
            warm()
            k_sb = project(eT, wk_sb, ENC_C, 1, "k")
            warm()
            v_sb = project(eT, wv_sb, ENC_C, 2, "v")

            # vT layout [128, HID_C, 130]:
            #   cols 0-47  slice0 v^T   48-63 zeros  64  ones
            #   cols 65-112 slice1 v^T  113-127 zeros 128 ones  129 zero
            # The ones column folds the attn column-sum into the v@attn
            # matmul (psum row 64 = sum over c of attn chunk).
            vT = work.tile([P, HID_C, 130], MMDT, tag="T_v")
            for g0 in range(0, HID_C, 4):
                pt = ps_t.tile([P, 4, PW], TDT, tag="pst")
                for i in range(4):
                    nc.tensor.transpose(
                        pt[:, i, :], v_sb.bitcast(TDT)[:, ts(g0 + i, P)],
                        ident_t[:PW, :PW],
                    )
                nc.scalar.copy(out=vT[:, :, 0:64], in_=pt[:, :, 0:64])
                nc.scalar.copy(out=vT[:, :, 65:113], in_=pt[:, :, 64:112])
            nc.scalar.copy(
                out=vT[:, :, 64:65],
                in_=ones_cf.rearrange("p (a b) -> p a b", a=4),
            )
            nc.scalar.copy(
                out=vT[:, :, 129:130],
                in_=ones_cf.rearrange("p (a b) -> p a b", a=4),
            )
            nc.scalar.copy(
                out=vT[:, :, 113:129],
                in_=zeros_t[:, :64].rearrange("p (a b) -> p a b", a=4),
            )

            # attn logits + exp for both slices, interleaved so the two
            # K=48 matmuls land in disjoint PE row groups and overlap.
            E_pair = [
                epool.tile([P, HID_C, HID], MMDT, tag="E0", name="E0"),
                epool.tile([P, HID_C, HID], MMDT, tag="E1", name="E1"),
            ]
            for m in range(HID_C):
                for s in range(2):
                    sb = 64 * s
                    pa = ps_a.tile([P, HID], F32, tag="ps_attn")
                    nc.tensor.matmul(
                        pa, q_sb[ds(sb, 48), ts(m, P)],
                        k_sb[ds(sb, 48), :],
                        start=True, stop=True,
                    )
                    nc.scalar.activation(
                        out=E_pair[s][:, m, :], in_=pa,
                        func=mybir.ActivationFunctionType.Exp,
                    )
                warm()

            uT = work.tile([P, HID_C, PW], MMDT, tag="T_u")
            nc.scalar.copy(
                out=uT[:, :, 48:64],
                in_=zeros_t[:, :64].rearrange("p (a b) -> p a b", a=4),
            )
            for s in range(2):
                h = h0 + s
                sb = 64 * s
                E_sb = E_pair[s]
                # u_unnorm = v @ exp(A); psum row 64 = column sums of exp(A)
                pu = ps_big.tile([PW, HID], F32, tag="ps_big")
                for m in range(HID_C):
                    nc.tensor.matmul(
                        pu[:65], vT[:, m, ds(65 * s, 65)], E_sb[:, m, :],
                        start=(m == 0), stop=(m == HID_C - 1),
                    )
                warm()
                s_sb = small.tile([1, HID], MMDT, tag="s_sb")
                nc.vector.tensor_copy(out=s_sb, in_=pu[64:65])
                # r = 1/s broadcast to all partitions (for attn normalize)
                pr = ps_a.tile([P, HID], F32, tag="ps_attn")
                nc.tensor.matmul(
                    pr, ones_row[:1, :], s_sb, start=True, stop=True
                )
                r_sb = epool.tile([P, HID], F32, tag="r_sb")
                nc.vector.reciprocal_approx_fast(out=r_sb, in_=pr)
                # rT = 1/s with k on partitions (for u^T rescale): transpose
                # s — four [1,128] -> [128,1] PE transposes into one bank.
                pst_s = ps_t.tile([P, 4], TDT, tag="pst")
                for m in range(HID_C):
                    nc.tensor.transpose(
                        pst_s[:, m:m + 1],
                        s_sb.bitcast(TDT)[:, ts(m, P)], ident_t[:1, :1],
                    )
                rT = small.tile([P, 4], F32, tag="rT")
                nc.vector.reciprocal_approx_fast(out=rT, in_=pst_s.bitcast(F32))
                # normalize attn output (off the out2 critical path)
                for m in range(HID_C):
                    nc.any.tensor_tensor(
                        out=E_sb[:, m, :], in0=E_sb[:, m, :], in1=r_sb,
                        op=mybir.AluOpType.mult,
                    )
                nc.sync.dma_start(
                    _dview(attn_d[h].rearrange("(o p) n -> p o n", p=P)), E_sb
                )
                # u^T with per-partition 1/s rescale during PSUM evacuation
                u_s = work.tile([48, HID], TDT, tag=f"u_s{s}")
                nc.vector.tensor_copy(out=u_s, in_=pu[:48])
                ptu = ps_t.tile([P, 4, PW], TDT, tag="pst")
                for m in range(HID_C):
                    nc.tensor.transpose(
                        ptu[:, m, :48], u_s[:, ts(m, P)], ident_t[:48, :48]
                    )
                for m in range(HID_C):
                    nc.vector.tensor_scalar(
                        out=uT[:, m, ds(sb, 48)], in0=ptu[:, m, :48],
                        scalar1=rT[:, m:m + 1], scalar2=None,
                        op0=mybir.AluOpType.mult,
                    )

            po = ps_big.tile([PW, HID], F32, tag="ps_big")
            for m in range(HID_C):
                nc.tensor.matmul(
                    po, uT[:, m, :], wo_sb[:, m, :],
                    start=(m == 0), stop=(m == HID_C - 1) and not use_bias,
                )
            if use_bias:
                nc.tensor.matmul(
                    po, ones_row[:, :PW], bias_sb[:, 3, :],
                    start=False, stop=True,
                )
            warm()
            o_sb = work.tile([PW, HID], F32, tag="o_sb")
            nc.any.tensor_tensor(
                out=o_sb, in0=po, in1=x_t, op=mybir.AluOpType.add
            )
            nc.sync.dma_start(out_d[h0], o_sb[0:48])
            nc.sync.dma_start(out_d[h0 + 1], o_sb[64:112])

    nc.compile()
    return nc


def kernel(**inputs):
    global LAST_RESULTS
    hidden = np.asarray(inputs["hidden"], dtype=np.float32)
    enc = np.asarray(inputs["enc"], dtype=np.float32)
    ln_g = np.asarray(inputs["ln_g"], dtype=np.float32)
    ln_b = np.asarray(inputs["ln_b"], dtype=np.float32)
    eln_g = np.asarray(inputs["eln_g"], dtype=np.float32)
    eln_b = np.asarray(inputs["eln_b"], dtype=np.float32)
    Wq = np.asarray(inputs["Wq"], dtype=np.float32)
    bq = np.asarray(inputs["bq"], dtype=np.float32)
    Wk = np.asarray(inputs["Wk"], dtype=np.float32)
    bk = np.asarray(inputs["bk"], dtype=np.float32)
    Wv = np.asarray(inputs["Wv"], dtype=np.float32)
    bv = np.asarray(inputs["bv"], dtype=np.float32)
    Wo = np.asarray(inputs["Wo"], dtype=np.float32)
    bo = np.asarray(inputs["bo"], dtype=np.float32)

    scale = math.sqrt(1.0 / W)
    # fold LN affine + attn scale into the projections (exact for g=1, b=0)
    wq_eff = (ln_g[:, None] * Wq) * scale
    bq_eff = (ln_b @ Wq + bq) * scale
    wk_eff = eln_g[:, None] * Wk
    bk_eff = eln_b @ Wk + bk
    wv_eff = eln_g[:, None] * Wv
    bv_eff = eln_b @ Wv + bv
    biases = np.stack([bq_eff, bk_eff, bv_eff, bo]).astype(np.float32)

    nc = build_program(use_bias=bool(np.any(biases)))

    in_maps = []
    for b in range(N_CORES):
        in_maps.append({
            "hidden": np.ascontiguousarray(hidden[b]),
            "enc": np.ascontiguousarray(enc[b]),
            "wq": np.ascontiguousarray(wq_eff),
            "wk": np.ascontiguousarray(wk_eff),
            "wv": np.ascontiguousarray(wv_eff),
            "wo": np.ascontiguousarray(Wo),
            "biases": biases,
        })

    res = run_bass_kernel_spmd(
        nc, in_maps, core_ids=list(range(N_CORES)),
        trace=bool(os.environ.get("KERNEL_TRACE")),
    )
    LAST_RESULTS = res

    out = np.stack([res.results[b]["out"] for b in range(N_CORES)])
    attn = np.stack([res.results[b]["attn"] for b in range(N_CORES)])
    return out, attn


# revision 23
# speedup vs baseline: 1.9545x; 1.1183x over previous
"""Trainium2 Bass kernel for nn_LionCrossAttentionDimH.

Computes, per (b, h) slice (H treated as independent "heads"):
    x = LN(hidden); e = LN(enc)
    q = x@Wq + bq ; k = e@Wk + bk ; v = e@Wv + bv        [48, 512]
    attn = softmax((q^T k) * sqrt(1/48), axis=c)          [512, 512]
    out  = (v @ attn) @ Wo + bo + hidden                  [48, 512]
Returns (out, attn) with full shapes [8,48,48,512] and [8,48,512,512].

Sharding: data-parallel over batch B=8 -> 8 NeuronCores (SPMD, no
collectives). Weights replicated; LN gains and the attn scale are folded
into the projection weights on the host.

Layout: h-slices are processed in pairs. Pair tensors use 112 partition
rows with slice0 at rows 0-47 and slice1 at rows 64-111 (pad rows 48-63
zeroed) so every per-slice engine access starts at partition 0 or 64 —
hardware requires 32-aligned partition starts.

Softmax: attn kept natural ([c,k], c on partitions). exp on ScalarE.
Column sums are folded into the v@attn matmul via an extra ones-column in
the vT operand (psum row 64 collects the sums). 1/sum via the fast DVE
reciprocal; u^T is rescaled per-partition during PSUM evacuation, while
the attn output tile is normalized with tensor_tensor ops off the
critical path.
"""

import math
import os
import sys
from contextlib import ExitStack

for _p in ("/opt/trn_rl_repo",):
    if _p not in sys.path:
        sys.path.insert(0, _p)

import numpy as np

import concourse.bass as bass
import concourse.tile as tile
from concourse import bacc, mybir
from concourse.bass import ds, ts
from concourse.bass_utils import run_bass_kernel_spmd
from concourse.masks import make_identity

# ---- problem constants (hardcoded per contract) ----
B, H, W = 8, 48, 48
HID, ENC = 512, 768
N_CORES = 8
P = 128
HID_C = HID // P   # 4
ENC_C = ENC // P   # 6
EPS = 1e-5
PW = 112           # padded pair width (rows 0-47 slice0, 64-111 slice1)

F32 = mybir.dt.float32
F32R = mybir.dt.float32r
I32 = mybir.dt.int32
BF16 = mybir.dt.bfloat16

MM_MODE = os.environ.get("KERNEL_MM", "f32r")
MMDT = F32R if MM_MODE == "f32r" else F32
TDT = F32R if os.environ.get("KERNEL_TDT", "f32") == "f32r" else F32
WARM_EVERY = int(os.environ.get("KERNEL_WARM", "0"))  # emit bf16 warm-up MMs

LAST_RESULTS = None  # BassKernelResults of the most recent kernel() call


def _dview(ap):
    """Bitcast a DRAM-side fp32 AP to the matmul dtype (same bytes)."""
    if MM_MODE == "f32r":
        return ap.bitcast(F32R)
    return ap


def build_program(use_bias=False):
    nc = bacc.Bacc(
        "TRN2", target_bir_lowering=False, debug=False, num_devices=N_CORES
    )

    hid_d = nc.dram_tensor("hidden", [H, W, HID], F32, kind="ExternalInput").ap()
    enc_d = nc.dram_tensor("enc", [H, W, ENC], F32, kind="ExternalInput").ap()
    wq_d = nc.dram_tensor("wq", [HID, HID], F32, kind="ExternalInput").ap()
    wk_d = nc.dram_tensor("wk", [ENC, HID], F32, kind="ExternalInput").ap()
    wv_d = nc.dram_tensor("wv", [ENC, HID], F32, kind="ExternalInput").ap()
    wo_d = nc.dram_tensor("wo", [HID, HID], F32, kind="ExternalInput").ap()
    bias_d = nc.dram_tensor("biases", [4, HID], F32, kind="ExternalInput").ap()

    out_d = nc.dram_tensor("out", [H, W, HID], F32, kind="ExternalOutput").ap()
    attn_d = nc.dram_tensor("attn", [H, HID, HID], F32, kind="ExternalOutput").ap()

    with tile.TileContext(nc) as tc, ExitStack() as ctx:
        singles = ctx.enter_context(tc.tile_pool(name="singles", bufs=1))
        io = ctx.enter_context(tc.tile_pool(name="io", bufs=3))
        work = ctx.enter_context(tc.tile_pool(name="work", bufs=2))
        epool = ctx.enter_context(tc.tile_pool(name="epool", bufs=2))
        small = ctx.enter_context(tc.tile_pool(name="small", bufs=4))
        stats = ctx.enter_context(tc.tile_pool(name="stats", bufs=4))
        ps_t = ctx.enter_context(tc.tile_pool(name="ps_t", bufs=2, space="PSUM"))
        ps_big = ctx.enter_context(tc.tile_pool(name="ps_big", bufs=3, space="PSUM"))
        ps_a = ctx.enter_context(tc.tile_pool(name="ps_a", bufs=2, space="PSUM"))
        ps_w = ctx.enter_context(tc.tile_pool(name="ps_w", bufs=1, space="PSUM"))

        # ---- constants / weights resident in SBUF ----
        ident = singles.tile([P, P], F32)
        make_identity(nc, ident)
        ident_t = singles.tile([P, P], TDT)
        nc.scalar.copy(out=ident_t, in_=ident)
        ones_rf = singles.tile([1, P], F32)
        nc.vector.memset(ones_rf, 1.0)
        ones_row = singles.tile([1, P], MMDT)
        nc.scalar.copy(out=ones_row, in_=ones_rf)
        ones_cf = singles.tile([P, 4], F32)
        nc.vector.memset(ones_cf, 1.0)
        zeros_t = singles.tile([P, 68], F32)
        nc.vector.memset(zeros_t, 0.0)
        # bf16 tiles for HAM warm-up matmuls (bf16 MMs count as PE activity
        # for the clock gate; fp32/f32r modes do not, leaving PE at 1.2 GHz)
        wa = singles.tile([P, 8], BF16)
        nc.vector.memset(wa, 1.0)
        wb = singles.tile([P, 64], BF16)
        nc.vector.memset(wb, 1.0)
        warm_ps = ps_w.tile([8, 64], F32, tag="warm")

        def warm():
            if WARM_EVERY:
                nc.tensor.matmul(warm_ps, wa, wb, start=True, stop=True)

        wq_sb = singles.tile([P, HID_C, HID], MMDT)
        nc.sync.dma_start(wq_sb, _dview(wq_d.rearrange("(o p) n -> p o n", p=P)))
        wk_sb = singles.tile([P, ENC_C, HID], MMDT)
        nc.sync.dma_start(wk_sb, _dview(wk_d.rearrange("(o p) n -> p o n", p=P)))
        wv_sb = singles.tile([P, ENC_C, HID], MMDT)
        nc.sync.dma_start(wv_sb, _dview(wv_d.rearrange("(o p) n -> p o n", p=P)))
        wo_sb = singles.tile([P, HID_C, HID], MMDT)
        nc.sync.dma_start(wo_sb, _dview(wo_d.rearrange("(o p) n -> p o n", p=P)))
        bias_sb = singles.tile([1, 4, HID], MMDT)
        nc.sync.dma_start(bias_sb, _dview(bias_d[None, :, :]))

        def load_pair(dram, h0, d, tag):
            """Load dram[h0] -> rows 0-47, dram[h0+1] -> rows 64-111."""
            t = io.tile([PW, d], F32, tag=tag)
            nc.gpsimd.memset(t[32:64], 0.0)
            nc.sync.dma_start(t[0:48], dram[h0])
            nc.sync.dma_start(t[64:112], dram[h0 + 1])
            return t

        def rsqrt_dve(va, tag):
            """1/sqrt(va) entirely on VectorE (bit-trick seed + 2 NR steps).

            Avoids Sqrt/Ln on ScalarE so the only ACT table set ever loaded
            is the Exp one (a table swap costs ~2.7us and was thrashing).
            """
            # seed = 0x5F3759DF - (bits >> 1), built from same-class ALU ops:
            # (bits>>1) ^ 0xFFFFFFFF == -(bits>>1) - 1, then add magic+1.
            sh = stats.tile([PW, 1], I32, tag=f"sh_{tag}")
            nc.vector.tensor_scalar(
                out=sh, in0=va.bitcast(I32), scalar1=1, scalar2=-1,
                op0=mybir.AluOpType.logical_shift_right,
                op1=mybir.AluOpType.bitwise_xor,
            )
            y = stats.tile([PW, 1], I32, tag=f"seed_{tag}")
            nc.vector.tensor_scalar(
                out=y, in0=sh, scalar1=0x5F3759DF + 1, scalar2=None,
                op0=mybir.AluOpType.add,
            )
            y = y.bitcast(F32)
            for it in range(2):
                t1 = stats.tile([PW, 1], F32, tag=f"nr1_{tag}{it}")
                nc.vector.tensor_tensor(
                    out=t1, in0=va, in1=y, op=mybir.AluOpType.mult
                )
                nc.vector.tensor_tensor(
                    out=t1, in0=t1, in1=y, op=mybir.AluOpType.mult
                )
                nc.vector.tensor_scalar(
                    out=t1, in0=t1, scalar1=-0.5, scalar2=1.5,
                    op0=mybir.AluOpType.mult, op1=mybir.AluOpType.add,
                )
                y2 = stats.tile([PW, 1], F32, tag=f"nr2_{tag}{it}")
                nc.vector.tensor_tensor(
                    out=y2, in0=y, in1=t1, op=mybir.AluOpType.mult
                )
                y = y2
            return y

        def layernorm(x_t, d, tag):
            """x_t: [PW, d] raw input -> returns normalized [PW, d] tile."""
            if d <= 512:
                st = stats.tile([PW, 6], F32, tag=f"st_{tag}")
                nc.vector.bn_stats(out=st, in_=x_t)
            else:
                gs = math.gcd(512, d)
                ng = d // gs
                st = stats.tile([PW, ng, 6], F32, tag=f"st_{tag}")
                for i in range(ng):
                    nc.vector.bn_stats(out=st[:, i, :], in_=x_t[:, ts(i, gs)])
            mv = stats.tile([PW, 2], F32, tag=f"mv_{tag}")
            nc.vector.bn_aggr(out=mv, in_=st)
            va = stats.tile([PW, 1], F32, tag=f"va_{tag}")
            nc.vector.tensor_scalar(
                out=va, in0=mv[:, 1:2], scalar1=EPS, scalar2=None,
                op0=mybir.AluOpType.add,
            )
            rstd = rsqrt_dve(va, tag)
            nmr = stats.tile([PW, 1], F32, tag=f"nmr_{tag}")
            nc.vector.tensor_tensor(
                out=nmr, in0=mv[:, 0:1], in1=rstd, op=mybir.AluOpType.mult
            )
            nc.vector.tensor_scalar(
                out=nmr, in0=nmr, scalar1=-1.0, scalar2=None,
                op0=mybir.AluOpType.mult,
            )
            x_ln = work.tile([PW, d], TDT, tag=f"ln_{tag}")
            nc.scalar.activation(
                out=x_ln, in_=x_t, func=mybir.ActivationFunctionType.Identity,
                bias=nmr, scale=rstd,
            )
            return x_ln

        def transpose_pair(x_ln, nchunk, tag, dt=None):
            """[PW, 128*nchunk] -> [128, nchunk, PW] via PE transposes.

            Up to 4 transposed chunks share one PSUM bank (4*112*4B < 2KB)
            and are evacuated with a single copy.
            """
            xT = work.tile([P, nchunk, PW], dt or MMDT, tag=f"T_{tag}")
            for g0 in range(0, nchunk, 4):
                gn = min(4, nchunk - g0)
                pt = ps_t.tile([P, 4, PW], TDT, tag="pst")
                for i in range(gn):
                    nc.tensor.transpose(
                        pt[:, i, :], x_ln[:, ts(g0 + i, P)], ident_t[:PW, :PW]
                    )
                nc.scalar.copy(
                    out=xT[:, g0:g0 + gn, :], in_=pt[:, :gn, :]
                )
            return xT

        def project(xT, w_sb, nchunk, bias_idx, tag):
            """pair projection: [PW,512] tile = xT.T @ W (+ bias)."""
            pq = ps_big.tile([PW, HID], F32, tag="ps_big")
            for m in range(nchunk):
                nc.tensor.matmul(
                    pq, xT[:, m, :], w_sb[:, m, :],
                    start=(m == 0), stop=(m == nchunk - 1) and not use_bias,
                )
            if use_bias:
                nc.tensor.matmul(
                    pq, ones_row[:, :PW], bias_sb[:, bias_idx, :],
                    start=False, stop=True,
                )
            o = work.tile([PW, HID], MMDT, tag=f"proj_{tag}")
            nc.scalar.copy(out=o, in_=pq)
            return o

        for j in range(H // 2):
            h0 = 2 * j
            x_t = load_pair(hid_d, h0, HID, "x_t")
            e_t = load_pair(enc_d, h0, ENC, "e_t")

            x_ln = layernorm(x_t, HID, "x")
            e_ln = layernorm(e_t, ENC, "e")
            xT = transpose_pair(x_ln, HID_C, "x")
            warm()
            eT = transpose_pair(e_ln, ENC_C, "e")

            q_sb = project(xT, wq_sb, HID_C, 0, "q")
            warm()
            k_sb = project(eT, wk_sb, ENC_C, 1, "k")
            warm()
            v_sb = project(eT, wv_sb, ENC_C, 2, "v")

            # vT layout [128, HID_C, 130]:
            #   cols 0-47  slice0 v^T   48-63 zeros  64  ones
            #   cols 65-112 slice1 v^T  113-127 zeros 128 ones  129 zero
            # The ones column folds the attn column-sum into the v@attn
            # matmul (psum row 64 = sum over c of attn chunk).
            vT = work.tile([P, HID_C, 130], MMDT, tag="T_v")
            for g0 in range(0, HID_C, 4):
                pt = ps_t.tile([P, 4, PW], TDT, tag="pst")
                for i in range(4):
                    nc.tensor.transpose(
                        pt[:, i, :], v_sb.bitcast(TDT)[:, ts(g0 + i, P)],
                        ident_t[:PW, :PW],
                    )
                nc.scalar.copy(out=vT[:, :, 0:64], in_=pt[:, :, 0:64])
                nc.scalar.copy(out=vT[:, :, 65:113], in_=pt[:, :, 64:112])
            nc.scalar.copy(
                out=vT[:, :, 64:65],
                in_=ones_cf.rearrange("p (a b) -> p a b", a=4),
            )
            nc.scalar.copy(
                out=vT[:, :, 129:130],
                in_=ones_cf.rearrange("p (a b) -> p a b", a=4),
            )
            nc.scalar.copy(
                out=vT[:, :, 113:129],
                in_=zeros_t[:, :64].rearrange("p (a b) -> p a b", a=4),
            )

            # attn logits + exp for both slices, interleaved so the two
            # K=48 matmuls land in disjoint PE row groups and overlap.
            E_pair = [
                epool.tile([P, HID_C, HID], MMDT, tag="E0", name="E0"),
                epool.tile([P, HID_C, HID], MMDT, tag="E1", name="E1"),
            ]
            for m in range(HID_C):
                for s in range(2):
                    sb = 64 * s
                    pa = ps_a.tile([P, HID], F32, tag="ps_attn")
                    nc.tensor.matmul(
                        pa, q_sb[ds(sb, 48), ts(m, P)],
                        k_sb[ds(sb, 48), :],
                        start=True, stop=True,
                    )
                    nc.scalar.activation(
                        out=E_pair[s][:, m, :], in_=pa,
                        func=mybir.ActivationFunctionType.Exp,
                    )
                warm()

            uT = work.tile([P, HID_C, PW], MMDT, tag="T_u")
            nc.scalar.copy(
                out=uT[:, :, 48:64],
                in_=zeros_t[:, :64].rearrange("p (a b) -> p a b", a=4),
            )
            for s in range(2):
                h = h0 + s
                sb = 64 * s
                E_sb = E_pair[s]
                # u_unnorm = v @ exp(A); psum row 64 = column sums of exp(A)
                pu = ps_big.tile([PW, HID], F32, tag="ps_big")
                for m in range(HID_C):
                    nc.tensor.matmul(
                        pu[:65], vT[:, m, ds(65 * s, 65)], E_sb[:, m, :],
                        start=(m == 0), stop=(m == HID_C - 1),
                    )
                warm()
                # u^T path first: it feeds the PE (uT transposes + out2)
                u_s = work.tile([48, HID], TDT, tag=f"u_s{s}")
                nc.scalar.copy(out=u_s, in_=pu[:48])
                s_sb = small.tile([1, HID], MMDT, tag="s_sb")
                nc.vector.tensor_copy(out=s_sb, in_=pu[64:65])
                ptu = ps_t.tile([P, 4, PW], TDT, tag="pst")
                for m in range(HID_C):
                    nc.tensor.transpose(
                        ptu[:, m, :48], u_s[:, ts(m, P)], ident_t[:48, :48]
                    )
                # rT = 1/s with k on partitions (for u^T rescale): transpose
                # s — four [1,128] -> [128,1] PE transposes into one bank.
                pst_s = ps_t.tile([P, 4], TDT, tag="pst")
                for m in range(HID_C):
                    nc.tensor.transpose(
                        pst_s[:, m:m + 1],
                        s_sb.bitcast(TDT)[:, ts(m, P)], ident_t[:1, :1],
                    )
                rT = small.tile([P, 4], F32, tag="rT")
                nc.vector.reciprocal_approx_fast(out=rT, in_=pst_s.bitcast(F32))
                for m in range(HID_C):
                    nc.vector.tensor_scalar(
                        out=uT[:, m, ds(sb, 48)], in0=ptu[:, m, :48],
                        scalar1=rT[:, m:m + 1], scalar2=None,
                        op0=mybir.AluOpType.mult,
                    )
                # r = 1/s broadcast to all partitions (for attn normalize)
                pr = ps_a.tile([P, HID], F32, tag="ps_attn")
                nc.tensor.matmul(
                    pr, ones_row[:1, :], s_sb, start=True, stop=True
                )
                r_sb = epool.tile([P, HID], F32, tag="r_sb")
                nc.vector.reciprocal_approx_fast(out=r_sb, in_=pr)
                # normalize attn output (off the out2 critical path)
                for m in range(HID_C):
                    nc.any.tensor_tensor(
                        out=E_sb[:, m, :], in0=E_sb[:, m, :], in1=r_sb,
                        op=mybir.AluOpType.mult,
                    )
                nc.sync.dma_start(
                    _dview(attn_d[h].rearrange("(o p) n -> p o n", p=P)), E_sb
                )

            po = ps_big.tile([PW, HID], F32, tag="ps_big")
            for m in range(HID_C):
                nc.tensor.matmul(
                    po, uT[:, m, :], wo_sb[:, m, :],
                    start=(m == 0), stop=(m == HID_C - 1) and not use_bias,
                )
            if use_bias:
                nc.tensor.matmul(
                    po, ones_row[:, :PW], bias_sb[:, 3, :],
                    start=False, stop=True,
                )
            warm()
            o_sb = work.tile([PW, HID], F32, tag="o_sb")
            nc.any.tensor_tensor(
                out=o_sb, in0=po, in1=x_t, op=mybir.AluOpType.add
            )
            nc.sync.dma_start(out_d[h0], o_sb[0:48])
            nc.sync.dma_start(out_d[h0 + 1], o_sb[64:112])

    nc.compile()
    return nc


def kernel(**inputs):
    global LAST_RESULTS
    hidden = np.asarray(inputs["hidden"], dtype=np.float32)
    enc = np.asarray(inputs["enc"], dtype=np.float32)
    ln_g = np.asarray(inputs["ln_g"], dtype=np.float32)
    ln_b = np.asarray(inputs["ln_b"], dtype=np.float32)
    eln_g = np.asarray(inputs["eln_g"], dtype=np.float32)
    eln_b = np.asarray(inputs["eln_b"], dtype=np.float32)
    Wq = np.asarray(inputs["Wq"], dtype=np.float32)
    bq = np.asarray(inputs["bq"], dtype=np.float32)
    Wk = np.asarray(inputs["Wk"], dtype=np.float32)
    bk = np.asarray(inputs["bk"], dtype=np.float32)
    Wv = np.asarray(inputs["Wv"], dtype=np.float32)
    bv = np.asarray(inputs["bv"], dtype=np.float32)
    Wo = np.asarray(inputs["Wo"], dtype=np.float32)
    bo = np.asarray(inputs["bo"], dtype=np.float32)

    scale = math.sqrt(1.0 / W)
    # fold LN affine + attn scale into the projections (exact for g=1, b=0)
    wq_eff = (ln_g[:, None] * Wq) * scale
    bq_eff = (ln_b @ Wq + bq) * scale
    wk_eff = eln_g[:, None] * Wk
    bk_eff = eln_b @ Wk + bk
    wv_eff = eln_g[:, None] * Wv
    bv_eff = eln_b @ Wv + bv
    biases = np.stack([bq_eff, bk_eff, bv_eff, bo]).astype(np.float32)

    nc = build_program(use_bias=bool(np.any(biases)))

    in_maps = []
    for b in range(N_CORES):
        in_maps.append({
            "hidden": np.ascontiguousarray(hidden[b]),
            "enc": np.ascontiguousarray(enc[b]),
            "wq": np.ascontiguousarray(wq_eff),
            "wk": np.ascontiguousarray(wk_eff),
            "wv": np.ascontiguousarray(wv_eff),
            "wo": np.ascontiguousarray(Wo),
            "biases": biases,
        })

    res = run_bass_kernel_spmd(
        nc, in_maps, core_ids=list(range(N_CORES)),
        trace=bool(os.environ.get("KERNEL_TRACE")),
    )
    LAST_RESULTS = res

    out = np.stack([res.results[b]["out"] for b in range(N_CORES)])
    attn = np.stack([res.results[b]["attn"] for b in range(N_CORES)])
    return out, attn
